# revision 1
# baseline (speedup 1.0000x reference)
"""DCN cross-layer (3 layers) + sync BatchNorm on 8 trn2 cores.

Math (per layer i, x0 = input):
    s_i[b]  = sum_d out_{i-1}[b,d] * w_i[d]
    y_i     = out_{i-1} + x0 * s_i[:,None] + bias_i          (bias cancels in BN)
    out_i   = (y_i - mean_b y_i) * rsqrt(var_b y_i + eps) * gamma_i + beta_i

Define z_i = out_{i-1} + x0*s_i (y_i minus the bias term). Since bias_i is
constant over the batch, BN(y_i) == (z_i - mean z_i)*rsqrt(var z_i+eps)*g + b.
So out_i = z_i*A_i + C_i with A_i = gamma_i*rsqrt(var+eps), C_i = beta_i - mean*A_i.

Device layout: z kept resident in SBUF in D-major layout [128 dpart, 8 chunks, Bloc]
as float32r. Per layer:
  u-pass (gpsimd): z <- z*A + C in place (materializes out_{i-1})
  s-MMs (PE):      psum_row[1,Nblk] = sum_c w[:,c]^T @ z[:,c,blk]   (fp32r)
  reshape (DMA):   s row -> sv[128, rg] per-partition scalars
  t-pass (DVE):    t_rows[128b,1024d] = x0_rows * sv[rg]            (fp32r out)
  matvec (PE):     psum_sum[1,1024] += ones^T @ t_rows  (= sum_b t, per d)
  transpose (PE):  t_rows -> psum_t[128d, 8c, 128b]                  (fp32r)
  z-update (DVE):  z[:,:,rgcols] = psum_t + z  (STT, in place)
  squares (ACT):   Square(z) with accum -> sum z^2 per chunk
  stats:           AllReduce (sum_t, sum_zsq); sumz_g recurrence;
                   A_i, C_i from mean/var.
Layer 1 has no u-pass; s_1 comes from a gpsimd STT over x0 rows (B-layout),
and z_1 = x0*(1+s_1) via one DVE STT.
Output: z*A_3+C_3 (gpsimd, in place), PE-transpose back to rows, DMA out.
"""
import numpy as np
import concourse.bass as bass
import concourse.bacc as bacc
import concourse.tile as tile
from concourse import mybir, masks

F32 = mybir.dt.float32
F32R = mybir.dt.float32r
ADD = mybir.AluOpType.add
MULT = mybir.AluOpType.mult
SUB = mybir.AluOpType.subtract
AX = mybir.AxisListType.X
AF = mybir.ActivationFunctionType

P = 128
D = 1024
C = D // P      # 8 d-chunks
L = 3
BN_EPS = 1e-5


def build_nc(n_cores: int, b_loc: int, b_total: int, debug: bool = False,
             repeats: int = 1, sim_mode: bool = False):
    NBLK = 512                 # columns per s-MM block
    nblk = b_loc // NBLK       # col blocks per core
    nrg = b_loc // P           # row groups (128 batch rows each)
    rg_per_blk = NBLK // P

    nc = bacc.Bacc("TRN2", target_bir_lowering=False, debug=False,
                   num_devices=n_cores)
    inp = nc.dram_tensor("inp", [b_loc, D], F32, kind="ExternalInput")
    weights = nc.dram_tensor("weights", [L, D], F32, kind="ExternalInput")
    gammas = nc.dram_tensor("gammas", [L, D], F32, kind="ExternalInput")
    betas = nc.dram_tensor("betas", [L, D], F32, kind="ExternalInput")
    out = nc.dram_tensor("out", [b_loc, D], F32, kind="ExternalOutput")
    if debug:
        NRG = b_loc // P
        dbg_sv = nc.dram_tensor("dbg_sv", [P, L, NRG], F32, kind="ExternalOutput")
        dbg_stin = nc.dram_tensor("dbg_stin", [L, P, 2 * C], F32, kind="ExternalOutput")
        dbg_stg = nc.dram_tensor("dbg_stg", [L, P, 2 * C], F32, kind="ExternalOutput")
        dbg_A = nc.dram_tensor("dbg_A", [P, L, C], F32, kind="ExternalOutput")
        dbg_C = nc.dram_tensor("dbg_C", [P, L, C], F32, kind="ExternalOutput")

    nblk_ = b_loc // 512
    scr_sv = nc.dram_tensor("scr_sv", [L * repeats, nblk_, 512], F32,
                            kind="Internal")
    scr_sum = nc.dram_tensor("scr_sum", [L * repeats, D], F32, kind="Internal")
    cc_in = [nc.dram_tensor(f"cc_in{i}", [P, 2 * C], F32, kind="Internal")
             for i in range(L * repeats)]
    cc_out = [nc.dram_tensor(f"cc_out{i}", [P, 2 * C], F32, kind="Internal",
                             addr_space="Shared") for i in range(L * repeats)]

    with tile.TileContext(nc) as tc:
        with (
            tc.tile_pool(name="zpool", bufs=1) as zpool,
            tc.tile_pool(name="big", bufs=5) as big,       # x0 rows stream
            tc.tile_pool(name="tbuf", bufs=4) as tbuf,     # t rows
            tc.tile_pool(name="outbuf", bufs=3) as outbuf, # out rows
            tc.tile_pool(name="small", bufs=1) as small,   # persistent smalls
            tc.tile_pool(name="stats", bufs=2) as stats,
            tc.tile_pool(name="srow", bufs=2) as srowp,
            tc.tile_pool(name="sq", bufs=2) as sqp,
            tc.tile_pool(name="pst", bufs=2, space="PSUM") as pst,
            tc.tile_pool(name="psrow", bufs=2, space="PSUM") as psrow,
            tc.tile_pool(name="pssum", bufs=1, space="PSUM") as pssum,
        ):
            # ---------- static setup ----------
            ident32 = small.tile([P, P], F32)
            masks.make_identity(nc, ident32[:])
            ident = small.tile([P, P], F32R)
            nc.vector.tensor_copy(out=ident[:], in_=ident32[:])

            ones_col = small.tile([P, 1], F32R)
            ones_col32 = small.tile([P, 1], F32)
            nc.vector.memset(ones_col32[:], 1.0)
            nc.vector.tensor_copy(out=ones_col[:], in_=ones_col32[:])
            eps_t = small.tile([P, 1], F32)
            nc.vector.memset(eps_t[:], BN_EPS)

            # per-layer params in D-major [p, c] layout: elem (p,c) = vec[c*128+p]
            w_sb = small.tile([P, L, C], F32)
            g_sb = small.tile([P, L, C], F32)
            be_sb = small.tile([P, L, C], F32)
            for l in range(L):
                nc.sync.dma_start(
                    out=w_sb[:, l, :],
                    in_=weights.ap()[l, :].rearrange("(c p) -> p c", p=P))
                nc.sync.dma_start(
                    out=g_sb[:, l, :],
                    in_=gammas.ap()[l, :].rearrange("(c p) -> p c", p=P))
                nc.sync.dma_start(
                    out=be_sb[:, l, :],
                    in_=betas.ap()[l, :].rearrange("(c p) -> p c", p=P))
            w_r = small.tile([P, L, C], F32R)
            nc.vector.tensor_copy(out=w_r[:], in_=w_sb[:])

            # layer-1 weight row replicated across partitions (B-layout)
            w1_rep = small.tile([P, D], F32)
            nc.sync.dma_start(out=w1_rep[:],
                              in_=weights.ap()[0:1, :].to_broadcast([P, D]))

            # z resident [p, c, b_loc] fp32r
            z = zpool.tile([P, C, b_loc], F32R)

            # per-layer A (scale) and Cv (shift), sumz global recurrence
            A_t = small.tile([P, L, C], F32)
            Cv_t = small.tile([P, L, C], F32)
            sumz_g = small.tile([P, C], F32)
            garbage = small.tile([P, D], F32)   # gpsimd STT main output (unused)
            sv = small.tile([P, L, nrg], F32)   # per-rg scale scalars per layer
            sv1p = small.tile([P, nrg], F32)    # 1+s1 for layer 1

            def layer_sweep(li, cc_base):
                """One layer's main sweep over all row groups / blocks."""
                nsp = (nblk + 1) // 2     # square-accum spans (2 blocks each)
                sq_parts = sqp.tile([P, C, nsp], F32, tag="sqacc")
                sqscr = sqp.tile([P, 2 * NBLK], mybir.dt.bfloat16, tag="sqscr", bufs=1)
                sqscr2 = sqp.tile([P, 2 * NBLK], mybir.dt.bfloat16, tag="sqscr2", bufs=1)
                psum_sum_f = pssum.tile([P, D], F32, tag="psum")
                psum_sum = psum_sum_f[0:1, :]
                for j in range(nblk):
                    if li > 0:
                        # s-row for this block from PE over current z(=out rows)
                        pr_f = psrow.tile([P, NBLK], F32, tag="psrow")
                        pr = pr_f[0:1, :]
                        for c in range(C):
                            nc.tensor.matmul(
                                pr,
                                lhsT=w_r[:, li, c : c + 1],
                                rhs=z[:, c, j * NBLK : (j + 1) * NBLK],
                                start=(c == 0), stop=(c == C - 1),
                            )
                        sr = srowp.tile([1, NBLK], F32, tag="srow")
                        nc.scalar.copy(out=sr[:], in_=pr)
                        # reshape row -> per-partition scalars sv[:, rg-range]
                        # (SBUF partition dim is physical: bounce via DRAM)
                        nc.sync.dma_start(out=scr_sv.ap()[cc_base + li, j, :], in_=sr[0:1, :])
                        nc.sync.dma_start(
                            out=sv[:, li, j * rg_per_blk : (j + 1) * rg_per_blk],
                            in_=scr_sv.ap()[cc_base + li, j, :].rearrange("(r p) -> p r", p=P),
                        )
                    for r in range(rg_per_blk):
                        rg = j * rg_per_blk + r
                        x0r = big.tile([P, D], F32, tag="x0r")
                        nc.sync.dma_start(
                            out=x0r[:], in_=inp.ap()[rg * P : (rg + 1) * P, :])
                        tr = tbuf.tile([P, D], F32R, tag="trows")
                        if li == 0:
                            # s1 per-partition via DVE STT accumulate
                            nc.vector.scalar_tensor_tensor(
                                out=garbage[:], in0=x0r[:], scalar=0.0,
                                in1=w1_rep[:], op0=ADD, op1=MULT,
                                accum_out=sv[:, 0, rg : rg + 1],
                            )
                            # 1+s1 (tiny, gpsimd) then t = x0*(1+s1) (TS, 2x)
                            nc.gpsimd.tensor_scalar_add(
                                out=sv1p[:, rg : rg + 1],
                                in0=sv[:, 0, rg : rg + 1], scalar1=1.0,
                            )
                            nc.vector.tensor_scalar(
                                out=tr[:], in0=x0r[:],
                                scalar1=sv1p[:, rg : rg + 1], scalar2=None,
                                op0=MULT,
                            )
                        else:
                            nc.vector.tensor_scalar(
                                out=tr[:], in0=x0r[:],
                                scalar1=sv[:, li, rg : rg + 1], scalar2=None,
                                op0=MULT,
                            )
                        # column sums of t (per d) accumulated over all rgs
                        for h in range(2):
                            nc.tensor.matmul(
                                psum_sum[0:1, h * NBLK : (h + 1) * NBLK],
                                lhsT=ones_col[:],
                                rhs=tr[:, h * NBLK : (h + 1) * NBLK],
                                start=(rg == 0), stop=(rg == nrg - 1),
                                skip_group_check=True,
                            )
                        # transpose t into D-layout psum
                        pt = pst.tile([P, C, P], F32R, tag="ptile")
                        for c in range(C):
                            nc.tensor.transpose(
                                pt[:, c, :], tr[:, c * P : (c + 1) * P], ident[:])
                        # z update
                        zslice = z[:, :, rg * P : (rg + 1) * P]
                        if li == 0:
                            nc.scalar.copy(out=zslice, in_=pt[:])
                        else:
                            nc.vector.scalar_tensor_tensor(
                                out=zslice, in0=pt[:], scalar=0.0, in1=zslice,
                                op0=ADD, op1=ADD,
                            )
                    # inline squares: after every odd block (or final block),
                    # accumulate sum(z^2) for the 2-block span per chunk
                    if j % 2 == 1 or j == nblk - 1:
                        sp = j // 2
                        lo = (sp * 2) * NBLK
                        hi = (j + 1) * NBLK
                        last = j == nblk - 1
                        for c in range(C):
                            if last and c % 2 == 1:
                                # final span: split ACT/DVE to halve the
                                # serial layer-boundary tail
                                nc.vector.scalar_tensor_tensor(
                                    out=sqscr2[:, 0 : hi - lo],
                                    in0=z[:, c, lo:hi], scalar=0.0,
                                    in1=z[:, c, lo:hi], op0=ADD, op1=MULT,
                                    accum_out=sq_parts[:, c, sp : sp + 1],
                                )
                            else:
                                nc.scalar.activation(
                                    out=sqscr[:, 0 : hi - lo], in_=z[:, c, lo:hi],
                                    func=AF.Square,
                                    accum_out=sq_parts[:, c, sp : sp + 1],
                                )
                return sq_parts, psum_sum

            def layer_stats(li, cc_base, sq_parts, psum_sum):
                """Reduce partials, AllReduce, compute A_i, C_i."""
                # sum_t row -> [p, c]
                sumrow = srowp.tile([1, D], F32, tag="sumrow", bufs=1)
                nc.scalar.copy(out=sumrow[:], in_=psum_sum)
                # sum_t row -> cc_in[:, 0:C] directly via strided DRAM write
                # (out iterated c-major to match the row's (c p) element order)
                nc.sync.dma_start(
                    out=cc_in[cc_base + li].ap()[:, 0:C].rearrange("p c -> c p"),
                    in_=sumrow[0:1, :])
                # sum z^2 partials -> [p, c] -> cc_in[:, C:2C]
                sqred = stats.tile([P, C], F32, tag="sqred")
                nc.vector.tensor_reduce(
                    out=sqred[:], in_=sq_parts[:], axis=AX, op=ADD)
                nc.sync.dma_start(out=cc_in[cc_base + li].ap()[:, C : 2 * C],
                                  in_=sqred[:])
                if sim_mode:
                    nc.sync.dma_start(out=cc_out[cc_base + li].ap(),
                                      in_=cc_in[cc_base + li].ap())
                else:
                    nc.gpsimd.collective_compute(
                        "AllReduce", ADD,
                        replica_groups=[list(range(n_cores))],
                        ins=[cc_in[cc_base + li].ap().opt()],
                        outs=[cc_out[cc_base + li].ap().opt()],
                    )
                st_g = stats.tile([P, 2 * C], F32, tag="stg")
                nc.sync.dma_start(out=st_g[:], in_=cc_out[cc_base + li].ap())
                if debug:
                    nc.sync.dma_start(out=dbg_stg.ap()[li], in_=st_g[:])

                # global sum_z recurrence:
                #   li==0: sumz = sum_t
                #   else:  sumz = A_{li-1}*sumz_prev + B*C_{li-1} + sum_t
                if li == 0:
                    nc.vector.tensor_copy(out=sumz_g[:], in_=st_g[:, 0:C])
                else:
                    nc.vector.tensor_mul(
                        out=sumz_g[:], in0=sumz_g[:], in1=A_t[:, li - 1, :])
                    nc.vector.scalar_tensor_tensor(
                        out=sumz_g[:], in0=Cv_t[:, li - 1, :],
                        scalar=float(b_total), in1=sumz_g[:],
                        op0=MULT, op1=ADD,
                    )
                    nc.vector.tensor_add(
                        out=sumz_g[:], in0=sumz_g[:], in1=st_g[:, 0:C])
                # mean, var
                mean = stats.tile([P, C], F32, tag="mean")
                nc.vector.tensor_scalar_mul(
                    out=mean[:], in0=sumz_g[:], scalar1=1.0 / b_total)
                var = stats.tile([P, C], F32, tag="var")
                nc.vector.tensor_scalar_mul(
                    out=var[:], in0=st_g[:, C : 2 * C], scalar1=1.0 / b_total)
                msq = stats.tile([P, C], F32, tag="msq")
                nc.vector.tensor_mul(out=msq[:], in0=mean[:], in1=mean[:])
                nc.vector.tensor_sub(out=var[:], in0=var[:], in1=msq[:])
                # A = gamma * rsqrt(var+eps): sqrt then reciprocal
                nc.scalar.activation(out=var[:], in_=var[:], func=AF.Sqrt,
                                     bias=eps_t[:, 0:1])
                nc.vector.reciprocal(out=var[:], in_=var[:])
                nc.vector.tensor_mul(
                    out=A_t[:, li, :], in0=var[:], in1=g_sb[:, li, :])
                # C = beta - mean*A
                nc.vector.tensor_mul(out=msq[:], in0=mean[:],
                                     in1=A_t[:, li, :])
                nc.vector.tensor_sub(
                    out=Cv_t[:, li, :], in0=be_sb[:, li, :], in1=msq[:])

            for rep in range(repeats):
              cc_base = rep * L
              for li in range(L):
                if li > 0:
                    # u-pass: z <- z*A_{li-1} + C_{li-1}  (now z == out_{li-1})
                    # split chunks across DVE (2x mode), ACT and gpsimd to
                    # shorten the serial layer transition
                    for c in range(C):
                        if c < 4:
                            nc.vector.tensor_scalar(
                                out=z[:, c, :], in0=z[:, c, :],
                                scalar1=A_t[:, li - 1, c : c + 1],
                                scalar2=Cv_t[:, li - 1, c : c + 1],
                                op0=MULT, op1=ADD,
                            )
                        elif c < 7:
                            nc.scalar.activation(
                                out=z[:, c, :], in_=z[:, c, :],
                                func=AF.Identity,
                                scale=A_t[:, li - 1, c : c + 1],
                                bias=Cv_t[:, li - 1, c : c + 1],
                            )
                        else:
                            nc.gpsimd.tensor_scalar(
                                out=z[:, c, :], in0=z[:, c, :],
                                scalar1=A_t[:, li - 1, c : c + 1],
                                scalar2=Cv_t[:, li - 1, c : c + 1],
                                op0=MULT, op1=ADD,
                            )
                sq_parts, psum_sum = layer_sweep(li, cc_base)
                layer_stats(li, cc_base, sq_parts, psum_sum)

              if debug:
                nc.sync.dma_start(out=dbg_sv.ap(), in_=sv[:])
                nc.sync.dma_start(out=dbg_A.ap(), in_=A_t[:])
                nc.sync.dma_start(out=dbg_C.ap(), in_=Cv_t[:])
              # -------- output: out = z*A_3 + C_3, transpose back --------
              for c in range(C):
                if c < 4:
                    nc.vector.tensor_scalar(
                        out=z[:, c, :], in0=z[:, c, :],
                        scalar1=A_t[:, L - 1, c : c + 1],
                        scalar2=Cv_t[:, L - 1, c : c + 1],
                        op0=MULT, op1=ADD,
                    )
                elif c < 7:
                    nc.scalar.activation(
                        out=z[:, c, :], in_=z[:, c, :], func=AF.Identity,
                        scale=A_t[:, L - 1, c : c + 1],
                        bias=Cv_t[:, L - 1, c : c + 1],
                    )
                else:
                    nc.gpsimd.tensor_scalar(
                        out=z[:, c, :], in0=z[:, c, :],
                        scalar1=A_t[:, L - 1, c : c + 1],
                        scalar2=Cv_t[:, L - 1, c : c + 1],
                        op0=MULT, op1=ADD,
                    )
              for rg in range(nrg):
                po = pst.tile([P, C, P], F32R, tag="ptile")
                for c in range(C):
                    nc.tensor.transpose(
                        po[:, c, :], z[:, c, rg * P : (rg + 1) * P], ident[:])
                orow = outbuf.tile([P, D], F32, tag="orow")
                if rg % 2 == 0:
                    nc.scalar.copy(out=orow[:], in_=po[:])
                else:
                    nc.vector.tensor_scalar(out=orow[:], in0=po[:],
                                            scalar1=0.0, scalar2=None, op0=ADD)
                nc.sync.dma_start(
                    out=out.ap()[rg * P : (rg + 1) * P, :], in_=orow[:])
    nc.compile()
    return nc


def run(full_inputs: dict, n_cores: int = 8, debug: bool = False,
        repeats: int = 1):
    from concourse.bass_utils import run_bass_kernel_spmd
    inp = np.ascontiguousarray(full_inputs["inp"], dtype=np.float32)
    B = inp.shape[0]
    b_loc = B // n_cores
    nc = build_nc(n_cores, b_loc, B, debug=debug, repeats=repeats)
    w = np.ascontiguousarray(full_inputs["weights"], dtype=np.float32)
    g = np.ascontiguousarray(full_inputs["gammas"], dtype=np.float32)
    be = np.ascontiguousarray(full_inputs["betas"], dtype=np.float32)
    in_maps = [
        {"inp": inp[k * b_loc : (k + 1) * b_loc], "weights": w,
         "gammas": g, "betas": be}
        for k in range(n_cores)
    ]
    res = run_bass_kernel_spmd(nc, in_maps, core_ids=list(range(n_cores)))
    full = np.concatenate([res.results[k]["out"] for k in range(n_cores)], axis=0)
    if debug:
        return full, res
    return full



_NC_CACHE = {}


def kernel(inp, weights, biases, gammas, betas):
    """Full-input entry point: shards batch over 8 NeuronCores, runs the
    Bass kernel (sync-BN via AllReduce), gathers the full output.

    `biases` are accepted but unused on device: a per-feature constant added
    before a training-mode BatchNorm cancels exactly in (y - mean(y)).
    """
    n_cores = 8
    inp = np.ascontiguousarray(inp, dtype=np.float32)
    B = inp.shape[0]
    b_loc = B // n_cores
    key = (n_cores, b_loc, B)
    if key not in _NC_CACHE:
        _NC_CACHE[key] = build_nc(n_cores, b_loc, B)
    nc = _NC_CACHE[key]
    w = np.ascontiguousarray(weights, dtype=np.float32)
    g = np.ascontiguousarray(gammas, dtype=np.float32)
    be = np.ascontiguousarray(betas, dtype=np.float32)
    in_maps = [
        {"inp": inp[k * b_loc : (k + 1) * b_loc], "weights": w,
         "gammas": g, "betas": be}
        for k in range(n_cores)
    ]
    from concourse.bass_utils import run_bass_kernel_spmd
    res = run_bass_kernel_spmd(nc, in_maps, core_ids=list(range(n_cores)))
    out = np.concatenate([res.results[k]["out"] for k in range(n_cores)],
                         axis=0)
    return out.astype(np.float32)

